# revision 1
# baseline (speedup 1.0000x reference)
"""Trainium2 Bass kernel for nn_CapsuleLayer (dynamic routing capsule layer).

Sharding: data-parallel on batch B=64 across 8 NeuronCores (8 per core).
Per core: 1152 positions (b, h*w), tiled 9 x 128 positions on SBUF partitions.

Layout A: positions on partitions, features on free dim.
votes[p, (o,at,i)] via PE (x^T stationary, block-diagonal weights moving),
kept in bf16. Routing iterations use DVE tensor ops with pairwise-tree
reductions (TT-add runs 2x on bf16 where tensor_reduce is stuck at 1x),
ACT for PSUM evacuation / transcendental / square, and scalar_tensor_tensor
to fold the +bias into the squash multiply.
"""

import numpy as np

B, I, A, H, W = 64, 32, 8, 12, 12
HW = H * W                     # 144
O, AT = 10, 16
OAT = O * AT                   # 160
NCORES = 8
BL = B // NCORES               # 8 local batch
NPOS = BL * HW                 # 1152 positions per core
P = 128
NT = NPOS // P                 # 9 tiles
IA = I * A                     # 256
NV = I * OAT                   # 5120 votes per position
NUM_ROUTING = 3
BIAS_CONST = 0.1               # module bias init (verified at runtime)

_BUILD_CACHE = {}


def _split_multiwait_instructions(nc):
    """This walrus build accepts only ONE sync-wait per instruction.
    Hoist extra waits onto injected single-wait NoOps on the same engine,
    placed immediately before the instruction."""
    from concourse import mybir

    k = 0
    for f in nc.m.functions:
        for b in f.blocks:
            out = []
            changed = False
            for ins in b.instructions:
                si = ins.sync_info
                if si is not None and len(si.on_wait) > 1:
                    waits = list(si.on_wait)
                    for w in waits[:-1]:
                        k += 1
                        out.append(
                            mybir.InstNoOp(
                                name=f"mwsplit-{k}",
                                engine=ins.engine,
                                sync_info=mybir.SyncInfo(
                                    on_wait=[w], on_update=[]
                                ),
                                bass_nofuse=True,
                            )
                        )
                    ins.sync_info = mybir.SyncInfo(
                        on_wait=[waits[-1]], on_update=list(si.on_update)
                    )
                    changed = True
                out.append(ins)
            if changed:
                b.instructions = out


def _tile_segments(t):
    """Tile t covers flat positions [128t, 128t+128); split at b boundaries.
    Returns (partition_offset, b, hw0, hw1) segments."""
    segs = []
    p0 = 0
    flat = P * t
    end = flat + P
    while flat < end:
        b = flat // HW
        hw0 = flat - b * HW
        hw1 = min(HW, hw0 + (end - flat))
        segs.append((p0, b, hw0, hw1))
        p0 += hw1 - hw0
        flat += hw1 - hw0
    return segs


def _build_program():
    import concourse.bass as bass
    import concourse.tile as tile
    from concourse import mybir

    f32 = mybir.dt.float32
    bf16 = mybir.dt.bfloat16
    AX = mybir.AxisListType
    AF = mybir.ActivationFunctionType
    OP = mybir.AluOpType

    nc = bass.Bass("TRN2", debug=False)

    x_d = nc.dram_tensor("x", [BL, I, A, HW], bf16, kind="ExternalInput").ap()
    wbd_d = nc.dram_tensor("wbd", [2, P, 5 * 512], bf16, kind="ExternalInput").ap()
    wsum_d = nc.dram_tensor("wsum", [2, P, OAT], bf16, kind="ExternalInput").ap()
    w2_d = nc.dram_tensor("w2", [P, 5 * 512], bf16, kind="ExternalInput").ap()
    ident_d = nc.dram_tensor("ident", [P, P], f32, kind="ExternalInput").ap()
    identb_d = nc.dram_tensor("identb", [P, P], bf16, kind="ExternalInput").ap()
    out_d = nc.dram_tensor("out", [BL, OAT, HW], f32, kind="ExternalOutput").ap()

    def bcast(ap, dims, offset=0):
        return bass.AP(
            tensor=ap.tensor, offset=ap.offset + offset,
            ap=[list(ap.ap[0])] + [list(d) for d in dims],
        )

    with tile.TileContext(nc) as tc:
        with (
            tc.tile_pool(name="singles", bufs=1) as singles,
            tc.tile_pool(name="votes", bufs=2) as votesp,
            tc.tile_pool(name="big", bufs=2) as bigp,
            tc.tile_pool(name="tree", bufs=2) as treep,
            tc.tile_pool(name="small", bufs=4) as smallp,
            tc.tile_pool(name="xfer", bufs=2) as xferp,
            tc.tile_pool(name="psum", bufs=3, space="PSUM") as psump,
            tc.tile_pool(name="psumg", bufs=2, space="PSUM") as psumg,
            tc.tile_pool(name="psum2", bufs=1, space="PSUM") as psump2,
            tc.tile_pool(name="psum3", bufs=1, space="PSUM") as psump3,
        ):
            bias01 = singles.tile([P, 1], f32, name="bias01")
            nc.vector.memset(bias01[:], BIAS_CONST)
            ident = singles.tile([P, P], f32, name="ident")
            nc.sync.dma_start(ident[:], ident_d[:, :])
            identb = singles.tile([P, P], bf16, name="identb")
            nc.sync.dma_start(identb[:], identb_d[:, :])
            w2 = singles.tile([P, 5 * 512], bf16, name="w2")
            nc.sync.dma_start(w2[:], w2_d[:, :])
            wbd = [singles.tile([P, 5 * 512], bf16, name=f"wbd{p}") for p in range(2)]
            wsum = [singles.tile([P, OAT], bf16, name=f"wsum{p}") for p in range(2)]
            for p in range(2):
                nc.sync.dma_start(wbd[p][:], wbd_d[p])
                nc.sync.dma_start(wsum[p][:], wsum_d[p])
            # x^T [(i,a), pos]: 2 chunks of 128 partitions (i 0-15 / 16-31)
            xT = [singles.tile([P, NPOS], bf16, name=f"xT{k}") for k in range(2)]
            for i in range(I):
                k, il = divmod(i, 16)
                nc.sync.dma_start(
                    xT[k][8 * il : 8 * il + 8, :].rearrange(
                        "p (b hw) -> p b hw", hw=HW
                    ),
                    x_d[:, i].rearrange("b a hw -> a b hw"),
                )

            from concourse.tile import add_dep_helper

            S = 2                       # max position-tiles per super-tile
            supers = [list(range(s, min(s + S, NT))) for s in range(0, NT, S)]
            prev_last_evac = None
            for tiles in supers:
                Sv = len(tiles)

                # iter-0 preact matmuls first: the PE runs these before
                # the votes matmuls so the whole iter-0 DVE/ACT chain overlaps
                # votes production.
                ps0s = []
                for u, t in enumerate(tiles):
                    ps0 = psump2.tile([P, OAT], f32, tag=f"pre0_{u}")
                    for p in range(2):
                        nc.tensor.matmul(
                            ps0[:],
                            xT[p][:, P * t : P * (t + 1)], wsum[p][:],
                            start=(p == 0), stop=(p == 1),
                        )
                    ps0s.append(ps0)

                # x (layout A) [128, (u,i,a)] bf16 via PE transpose of xT
                x_sb = xferp.tile([P, Sv, I, A], bf16, tag="x")
                for u, t in enumerate(tiles):
                    for k in range(2):
                        xtp = psump3.tile([P, P], bf16, tag="tp")
                        nc.tensor.transpose(
                            xtp[:], xT[k][:, P * t : P * (t + 1)], identb[:]
                        )
                        nc.scalar.copy(
                            x_sb[:, u, 16 * k : 16 * (k + 1), :], xtp[:]
                        )

                # ---- votes [128, (u, o,at,i)] bf16; i = 16*pack + il ----
                votes = votesp.tile([P, Sv, NV], bf16, tag="votes")
                first_mm = None
                for u, t in enumerate(tiles):
                    for p in range(2):
                        lhsT = xT[p][:, P * t : P * (t + 1)]
                        for j in range(5):
                            ps = psump.tile([P, 512], f32, tag="vchunk")
                            mm = nc.tensor.matmul(
                                ps[:], lhsT, wbd[p][:, 512 * j : 512 * (j + 1)],
                                start=True, stop=True,
                            )
                            if first_mm is None:
                                first_mm = mm
                            ev = nc.scalar.copy(
                                bcast(votes, [[512, 2], [32, AT], [1, 16]],
                                      offset=u * NV + 1024 * j + 16 * p),
                                ps.rearrange(
                                    "p (o at il) -> p o at il", o=2, at=AT
                                ),
                            )
                if prev_last_evac is not None and first_mm is not None:
                    add_dep_helper(
                        first_mm.ins, prev_last_evac.ins, sync=False,
                        reason="order votes matmuls after prior super evac",
                    )
                prev_last_evac = ev

                logits = None       # [128, (u,o,i)] fp32
                act = None
                for it in range(NUM_ROUTING):
                    # ---- preact sums [128, (u,o,at)] fp32 ----
                    if it == 0:
                        pre_src = None
                    else:
                        # softmax over o: logits [128, (u,o,i)]
                        e = smallp.tile([P, Sv * O * I], bf16, tag="e")
                        nc.scalar.activation(e[:], logits[:], AF.Exp)
                        s = smallp.tile([P, Sv * I], f32, tag="s")
                        nc.vector.reduce_sum(
                            s[:], bcast(e, [[O * I, Sv], [1, I], [I, O]]),
                            axis=AX.X,
                        )
                        rs = smallp.tile([P, Sv * I], f32, tag="rs")
                        nc.vector.reciprocal(rs[:], s[:])
                        route = smallp.tile([P, Sv * O * I], bf16, tag="route")
                        nc.vector.tensor_mul(
                            route.rearrange("p (u o i) -> p u o i", u=Sv, o=O),
                            e.rearrange("p (u o i) -> p u o i", u=Sv, o=O),
                            bcast(rs, [[I, Sv], [0, O], [1, I]]),
                        )
                        # m2 = votes * route_bcast(at)  [2x bf16], per tile
                        m2 = bigp.tile([P, Sv, NV], bf16, tag="m2")
                        for u in range(Sv):
                            nc.vector.tensor_mul(
                                m2[:, u].rearrange(
                                    "p (o at i) -> p o at i", o=O, at=AT
                                ),
                                votes[:, u].rearrange(
                                    "p (o at i) -> p o at i", o=O, at=AT
                                ),
                                bcast(route, [[I, O], [0, AT], [1, I]],
                                      offset=u * O * I),
                            )
                        # i-tree: 32 -> ... -> 1 over all Sv tiles at once
                        src, width = m2, 32
                        while width > 2:
                            width //= 2
                            nxt = treep.tile([P, Sv * OAT * width], bf16,
                                             tag=f"it{width}")
                            nc.vector.tensor_add(
                                nxt.rearrange("p (s i) -> p s i", i=width),
                                bcast(src, [[2 * width, Sv * OAT], [1, width]]),
                                bcast(src, [[2 * width, Sv * OAT], [1, width]],
                                      offset=width),
                            )
                            src = nxt
                        pre_s = smallp.tile([P, Sv * OAT], f32, tag="pre_s")
                        nc.vector.tensor_add(
                            pre_s.rearrange("p (s i) -> p s i", i=1),
                            bcast(src, [[2, Sv * OAT], [1, 1]]),
                            bcast(src, [[2, Sv * OAT], [1, 1]], offset=1),
                        )
                        pre_src = pre_s[:]

                    # ---- squash: preb = pre + 0.1 and its square, all on
                    # DVE (an ACT round-trip here stalls the DVE pipeline) ----
                    preb = smallp.tile([P, Sv * OAT], f32, tag="preb")
                    if it == 0:
                        for u in range(Sv):
                            nc.vector.tensor_scalar_add(
                                preb[:, u * OAT : (u + 1) * OAT], ps0s[u][:],
                                BIAS_CONST,
                            )
                    else:
                        nc.vector.tensor_scalar_add(preb[:], pre_src,
                                                    BIAS_CONST)
                    sq = smallp.tile([P, Sv * OAT], f32, tag="sq")
                    nc.vector.tensor_mul(sq[:], preb[:], preb[:])
                    nsq = smallp.tile([P, Sv * O], f32, tag="nsq")
                    nc.vector.reduce_sum(
                        nsq[:], sq.rearrange("p (o at) -> p o at", at=AT),
                        axis=AX.X,
                    )
                    norm = smallp.tile([P, Sv * O], f32, tag="norm")
                    nc.scalar.sqrt(norm[:], nsq[:])
                    den = smallp.tile([P, Sv * O], f32, tag="den")
                    nc.vector.tensor_scalar_add(den[:], nsq[:], 1.0)
                    rden = smallp.tile([P, Sv * O], f32, tag="rden")
                    nc.vector.reciprocal(rden[:], den[:])
                    scl = smallp.tile([P, Sv * O], f32, tag="scl")
                    nc.vector.tensor_mul(scl[:], norm[:], rden[:])
                    last = it == NUM_ROUTING - 1
                    if last:
                        act = xferp.tile([P, Sv, OAT], f32, tag="act")
                        # act = preb * scale_bcast(at); transpose for the
                        # contiguous output DMA
                        nc.vector.tensor_mul(
                            act.rearrange("p u (o at) -> p (u o) at", at=AT),
                            bcast(preb, [[AT, Sv * O], [1, AT]]),
                            bcast(scl, [[1, Sv * O], [0, AT]]),
                        )
                        actTs = []
                        for u, t in enumerate(tiles):
                            tp1 = psump3.tile([P, P], f32, tag="tp")
                            nc.tensor.transpose(tp1[:], act[:, u, 0:P],
                                                ident[:])
                            tp2 = psump3.tile([32, P], f32, tag="tp")
                            nc.tensor.transpose(tp2[:], act[:, u, P:OAT],
                                                ident[:])
                            actT1 = xferp.tile([P, P], f32, tag=f"actT1_{u}")
                            nc.scalar.copy(actT1[:], tp1[:])
                            actT2 = xferp.tile([32, P], f32, tag=f"actT2_{u}")
                            nc.scalar.copy(actT2[:], tp2[:])
                            actTs.append((actT1, actT2))
                        break
                    # iters 0/1: transpose preb instead of act — scl is
                    # constant over atoms, so dist = scl * (sum_at votes*preb)
                    # and the transpose/g-matmul chain starts before the
                    # norm/scale computation instead of after it.
                    actTs = []
                    for u, t in enumerate(tiles):
                        tp1 = psump3.tile([P, P], f32, tag="tp")
                        nc.tensor.transpose(
                            tp1[:], preb[:, u * OAT : u * OAT + P], ident[:]
                        )
                        tp2 = psump3.tile([32, P], f32, tag="tp")
                        nc.tensor.transpose(
                            tp2[:], preb[:, u * OAT + P : (u + 1) * OAT],
                            ident[:],
                        )
                        actT1 = xferp.tile([P, P], bf16, tag=f"actT1_{u}")
                        nc.scalar.copy(actT1[:], tp1[:])
                        actT2 = xferp.tile([32, P], bf16, tag=f"actT2_{u}")
                        nc.scalar.copy(actT2[:], tp2[:])
                        actTs.append((actT1, actT2))

                    # ---- distances via PE: g_b = actT_b.T @ w2_b, then
                    # dist = sum_a x * g  (a-tree)  ----
                    h = bigp.tile([P, Sv, NV // 2], bf16, tag="m2")
                    for u in range(Sv):
                        actT1, actT2 = actTs[u]
                        for bb in range(5):
                            bp = (32 * bb) % P
                            lhsT = actT1[bp : bp + 32, :] \
                                if bb < 4 else actT2[:, :]
                            gps = psumg.tile([P, 512], f32, tag="g")
                            nc.tensor.matmul(
                                gps[:], lhsT,
                                w2[bp : bp + 32, 512 * bb : 512 * (bb + 1)],
                                start=True, stop=True,
                                tile_position=((32 * bb) % P, 0),
                            )
                            nc.vector.tensor_mul(
                                h[:, u, 512 * bb : 512 * (bb + 1)].rearrange(
                                    "p (o2 i a) -> p o2 i a", o2=2, i=I
                                ),
                                bcast(x_sb, [[0, 2], [A, I], [1, A]],
                                      offset=u * IA),
                                gps.rearrange(
                                    "p (o2 i a) -> p o2 i a", o2=2, i=I
                                ),
                            )
                    # a-tree: 8 -> 4 -> 2 -> 1 (segments (u,b,o2,i) = Sv*320)
                    ha4 = treep.tile([P, Sv * 1280], bf16, tag="it8")
                    nc.vector.tensor_add(
                        ha4.rearrange("p (s a) -> p s a", a=4),
                        bcast(h, [[8, Sv * 320], [1, 4]]),
                        bcast(h, [[8, Sv * 320], [1, 4]], offset=4),
                    )
                    ha2 = treep.tile([P, Sv * 640], bf16, tag="it4")
                    nc.vector.tensor_add(
                        ha2.rearrange("p (s a) -> p s a", a=2),
                        bcast(ha4, [[4, Sv * 320], [1, 2]]),
                        bcast(ha4, [[4, Sv * 320], [1, 2]], offset=2),
                    )
                    dist_t = smallp.tile([P, Sv * O * I], f32, tag="dist_t")
                    nc.vector.tensor_add(
                        dist_t.rearrange("p (s a) -> p s a", a=1),
                        bcast(ha2, [[2, Sv * 320], [1, 1]]),
                        bcast(ha2, [[2, Sv * 320], [1, 1]], offset=1),
                    )
                    dist = smallp.tile([P, Sv * O * I], f32, tag="dist")
                    nc.vector.tensor_mul(
                        bcast(dist, [[I, Sv * O], [1, I]]),
                        bcast(dist_t, [[I, Sv * O], [1, I]]),
                        bcast(scl, [[1, Sv * O], [0, I]]),
                    )
                    if logits is None:
                        logits = dist
                    else:
                        logits2 = smallp.tile([P, Sv * O * I], f32,
                                              tag="logits2")
                        nc.vector.tensor_add(logits2[:], logits[:], dist[:])
                        logits = logits2

                for u, t in enumerate(tiles):
                    actT1, actT2 = actTs[u]
                    for (p0, b, hw0, hw1) in _tile_segments(t):
                        n = hw1 - hw0
                        nc.sync.dma_start(
                            out_d[b, 0:P, hw0:hw1], actT1[:, p0 : p0 + n]
                        )
                        nc.sync.dma_start(
                            out_d[b, P:OAT, hw0:hw1], actT2[:, p0 : p0 + n]
                        )
    _split_multiwait_instructions(nc)
    return nc


def _get_program():
    if "nc" not in _BUILD_CACHE:
        _BUILD_CACHE["nc"] = _build_program()
    return _BUILD_CACHE["nc"]


def _host_weights(weights):
    import ml_dtypes

    w = np.asarray(weights, dtype=np.float32)        # [I, A, OAT]
    # moving operand, pack p covers i in [16p, 16p+16).
    # rows = (il2, a); cols = (o, at, il); nonzero iff il2 == il.
    wbd = np.zeros((2, P, 2560), dtype=np.float32)
    for p in range(2):
        for il in range(16):
            blk = w[16 * p + il].reshape(A, O, AT)           # [a, o, at]
            for o in range(O):
                wbd[p, 8 * il : 8 * il + 8,
                    o * 256 + il : o * 256 + 256 : 16] = blk[:, o, :]
    # dense sum-over-i weights * 0.1 (uniform initial route)
    ws = 0.1 * w.reshape(IA, OAT)
    wsum = np.stack([ws[:P], ws[P:]], axis=0)
    # per-o-pair distance weights: w2[(o2r,at), b*512 + o2*256 + i*8 + a]
    w2 = np.zeros((P, 5 * 512), dtype=np.float32)
    for bb in range(5):
        bp = (32 * bb) % P
        for o2 in range(2):
            o = 2 * bb + o2
            # [at, i, a] block
            blk = w.reshape(I, A, O, AT)[:, :, o, :].transpose(2, 0, 1)
            w2[bp + o2 * 16 : bp + o2 * 16 + 16,
               bb * 512 + o2 * 256 : bb * 512 + o2 * 256 + 256] = (
                blk.reshape(AT, IA)
            )
    return (
        wbd.astype(ml_dtypes.bfloat16),
        wsum.astype(ml_dtypes.bfloat16),
        w2.astype(ml_dtypes.bfloat16),
    )


def kernel(x, weights, bias):
    import ml_dtypes

    assert np.allclose(np.asarray(bias, dtype=np.float32), BIAS_CONST), (
        "kernel assumes the constant 0.1 capsule bias"
    )
    x = np.ascontiguousarray(np.asarray(x, dtype=np.float32))
    ident = np.eye(P, dtype=np.float32)
    identb = ident.astype(ml_dtypes.bfloat16)
    xbf = x.reshape(B, I, A, HW).astype(ml_dtypes.bfloat16)
    wbd, wsum, w2 = _host_weights(weights)

    from concourse import bass_utils

    nc = _get_program()
    in_maps = []
    for c in range(NCORES):
        in_maps.append(
            {"x": xbf[BL * c : BL * (c + 1)], "wbd": wbd, "wsum": wsum,
             "w2": w2, "ident": ident, "identb": identb}
        )
    res = bass_utils.run_bass_kernel_spmd(
        nc, in_maps, core_ids=list(range(NCORES))
    )
    out = np.concatenate([res.results[c]["out"] for c in range(NCORES)], axis=0)
    return out.reshape(B, O, AT, H, W).astype(np.float32)

